# revision 10
# baseline (speedup 1.0000x reference)
"""GCN layer (GraphConv + BN + dropout) as a Trainium2 Bass kernel, SPMD over 8 NeuronCores.

v2 design:
- dst-node sharding across 8 cores; gather table = feat * norm_src (folded on host,
  fp16) replicated per core; edges partitioned by (dst-tile, src-bank), padded to
  128-edge chunks with counts uniform across cores (max) so one NEFF runs SPMD.
- aggregation: dma_gather (4 SWDGE queues) + per-chunk 0/1 one-hot S (bf16, exact)
  matmul-accumulated into PSUM: aggT[f, d] += G^T S.
- stage B per tile: PSUM = invnorm_dst*b (K=1 matmul) + agg @ W, then
  h = relu(norm_dst * PSUM) via activation per-partition scale. BN stats via
  ones-matmul accumulation, all-reduced across cores; apply + dropout mask on DVE.
"""
import sys

sys.path.insert(0, "/opt/trn_rl_repo")

import numpy as np
import ml_dtypes

import concourse.bass as bass
import concourse.bacc as bacc
import concourse.mybir as mybir
import concourse.tile as tile
from concourse.bass_utils import run_bass_kernel_spmd

NCORES = 8
P = 128
BANK = 32768          # rows addressable by int16 gather index
OPC = 4               # max chunks (of 128 rows) per dma_gather op
NQ = 4                # SWDGE queues to rotate gather ops over
BN_EPS = 1e-5


def _host_prep(N, D, src, dst):
    """Partition edges by (core, dst-tile, src-bank); pad to 128-edge chunks with
    counts uniform across cores (max), producing per-core streams."""
    rpc = N // NCORES                      # rows per core
    NT = (rpc + P - 1) // P                # dst tiles per core
    NB = (N + BANK - 1) // BANK            # src banks

    deg_out = np.bincount(src, minlength=N).astype(np.float64)
    deg_in = np.bincount(dst, minlength=N).astype(np.float64)
    norm_src = (np.where(deg_out > 0, deg_out, 1.0) ** -0.5).astype(np.float32)
    norm_dst = (np.where(deg_in > 0, deg_in, 1.0) ** -0.5).astype(np.float32)

    core_of = dst // rpc
    ldst = dst - core_of * rpc
    tile_of = ldst // P
    rel_of = (ldst % P).astype(np.float32)
    bank_of = src >> 15
    lsrc = (src & (BANK - 1)).astype(np.int16)
    seg_of = tile_of * NB + bank_of        # segment id within a core

    NSEG = NT * NB
    per_core = []
    L = np.zeros((NCORES, NSEG), np.int64)
    for c in range(NCORES):
        m = core_of == c
        order = np.argsort(seg_of[m], kind="stable")
        idx = np.nonzero(m)[0][order]
        per_core.append((lsrc[idx], rel_of[idx]))
        L[c] = np.bincount(seg_of[m], minlength=NSEG)

    n_chunks = (L.max(axis=0) + P - 1) // P        # [NSEG], uniform over cores
    seg_pad = n_chunks * P
    total_edges = int(seg_pad.sum())
    TC = total_edges // P                          # total chunks per core

    ops = []  # (bank, nidx, chunk_base)
    k = 0
    for s in range(NSEG):
        nc_s = int(n_chunks[s])
        if nc_s == 0:
            continue
        b = s % NB
        done = 0
        while done < nc_s:
            take = min(OPC, nc_s - done)
            ops.append((b, take * P, k))
            k += take
            done += take
    assert k == TC

    idx_w = total_edges // 16
    idx_all = np.zeros((NCORES, 128, idx_w), np.int16)
    rel_all = np.full((NCORES, P, TC), -1.0, np.float32)
    seg_off = np.zeros(NSEG + 1, np.int64)
    np.cumsum(seg_pad, out=seg_off[1:])
    for c in range(NCORES):
        ls, rel = per_core[c]
        src_pad = np.zeros(total_edges, np.int16)
        rel_pad = np.full(total_edges, -1.0, np.float32)
        off = np.cumsum(np.concatenate([[0], L[c]]))[:-1]
        for s in range(NSEG):
            n = int(L[c, s])
            if n == 0:
                continue
            dstart = int(seg_off[s])
            src_pad[dstart : dstart + n] = ls[off[s] : off[s] + n]
            rel_pad[dstart : dstart + n] = rel[off[s] : off[s] + n]
        rel_all[c] = rel_pad.reshape(TC, P).T
        pos = 0
        for (_b, nidx, kb) in ops:
            seg = src_pad[kb * P : kb * P + nidx]
            blk = seg.reshape(-1, 16).T          # [16, nidx/16]
            idx_all[c, :, pos : pos + nidx // 16] = np.tile(blk, (8, 1))
            pos += nidx // 16
        assert pos == idx_w

    meta = dict(N=N, D=D, rpc=rpc, NT=NT, NB=NB, TC=TC, idx_w=idx_w,
                n_chunks=n_chunks, ops=ops, total_edges=total_edges)
    return meta, idx_all, rel_all, norm_src, norm_dst


def _build_program(meta, repeat_main=1, use_collective=True):
    N, D = meta["N"], meta["D"]
    NT, NB, TC = meta["NT"], meta["NB"], meta["TC"]
    idx_w, ops, n_chunks = meta["idx_w"], meta["ops"], meta["n_chunks"]
    rpad = NT * P
    nvalid_last = meta["rpc"] - (NT - 1) * P       # valid rows in last tile
    f32 = mybir.dt.float32
    bf16 = mybir.dt.bfloat16
    fp16 = mybir.dt.float16

    nc = bacc.Bacc("TRN2", num_devices=NCORES, num_swdge_queues=NQ)
    table = nc.declare_dram_parameter("table", [N, D], fp16, isOutput=False)
    idxs = nc.declare_dram_parameter("idxs", [128, idx_w], mybir.dt.int16, isOutput=False)
    drel = nc.declare_dram_parameter("drel", [P, TC], f32, isOutput=False)
    ndstT = nc.declare_dram_parameter("ndstT", [P, NT], f32, isOutput=False)
    ninv = nc.declare_dram_parameter("ninv", [NT, P], f32, isOutput=False)
    maskv = nc.declare_dram_parameter("maskv", [rpad, D], f32, isOutput=False)
    Wp = nc.declare_dram_parameter("W", [D, D], f32, isOutput=False)
    bp = nc.declare_dram_parameter("b", [D], f32, isOutput=False)
    gp = nc.declare_dram_parameter("gamma", [D], f32, isOutput=False)
    btp = nc.declare_dram_parameter("beta", [D], f32, isOutput=False)
    yout = nc.declare_dram_parameter("y", [rpad, D], f32, isOutput=True)

    cc_in = nc.dram_tensor("cc_in", [1, 2 * D], f32)
    cc_out = nc.dram_tensor("cc_out", [1, 2 * D], f32, addr_space="Shared")

    iota_np = np.tile(np.arange(P).astype(ml_dtypes.bfloat16), (P, 1))
    iota_dram = nc.inline_tensor(iota_np, name="iota_const")
    ones_last_np = np.zeros((P, 1), np.float32)
    ones_last_np[:nvalid_last] = 1.0
    ones_last_dram = nc.inline_tensor(ones_last_np, name="ones_last_const")

    ops_by_tile = [[] for _ in range(NT)]
    chunk_tile = np.repeat(np.arange(NT * NB) // NB, n_chunks)
    pos = 0
    for (b, nidx, kb) in ops:
        ops_by_tile[int(chunk_tile[kb])].append((b, nidx, kb, pos))
        pos += nidx // 16

    with tile.TileContext(nc) as tc:
        with tc.tile_pool(name="consts", bufs=1) as cpool, \
             tc.tile_pool(name="streams", bufs=1) as stpool, \
             tc.tile_pool(name="hbuf", bufs=1) as hpool, \
             tc.tile_pool(name="gat", bufs=8) as gpool, \
             tc.tile_pool(name="sel", bufs=8) as spool, \
             tc.tile_pool(name="aggsb", bufs=3) as apool, \
             tc.tile_pool(name="sq", bufs=2) as qpool, \
             tc.tile_pool(name="ybuf", bufs=4) as ypool, \
             tc.tile_pool(name="msk", bufs=4) as mpool, \
             tc.tile_pool(name="stat", bufs=1) as tpool, \
             tc.tile_pool(name="apsum", bufs=3, space="PSUM") as apsum, \
             tc.tile_pool(name="hpsum", bufs=2, space="PSUM") as hpsum, \
             tc.tile_pool(name="spsum", bufs=1, space="PSUM") as spsum:

            iota_sb = cpool.tile([P, P], bf16)
            nc.sync.dma_start(out=iota_sb[:], in_=iota_dram[:])
            W_sb = cpool.tile([D, D], f32)
            nc.sync.dma_start(out=W_sb[:], in_=Wp[:])
            b_row = cpool.tile([1, D], f32)
            nc.sync.dma_start(out=b_row[:], in_=bp[None, :])
            g_row = cpool.tile([1, D], f32)
            nc.sync.dma_start(out=g_row[:], in_=gp[None, :])
            bt_row = cpool.tile([1, D], f32)
            nc.sync.dma_start(out=bt_row[:], in_=btp[None, :])
            ones_col = cpool.tile([P, 1], f32)
            nc.vector.memset(ones_col[:], 1.0)
            ones_last = cpool.tile([P, 1], f32)
            nc.sync.dma_start(out=ones_last[:], in_=ones_last_dram[:])
            ones_row = cpool.tile([1, P], f32)
            nc.vector.memset(ones_row[:], 1.0)

            idx_sb = stpool.tile([128, idx_w], mybir.dt.int16)
            nc.sync.dma_start(out=idx_sb[:], in_=idxs[:])
            drel_sb = stpool.tile([P, TC], f32)
            nc.sync.dma_start(out=drel_sb[:], in_=drel[:])
            ndstT_sb = stpool.tile([P, NT], f32)
            nc.sync.dma_start(out=ndstT_sb[:], in_=ndstT[:])
            ninv_sb = stpool.tile([1, NT * P], f32)
            nc.sync.dma_start(out=ninv_sb[:], in_=ninv[:].rearrange("t p -> (t p)")[None, :])

            h_sb = hpool.tile([P, rpad], f32)

            ssum_ps = spsum.tile([1, D], f32, tag="ssum", name="ssum")
            ssq_ps = spsum.tile([1, D], f32, tag="ssq", name="ssq")

            opi = 0
            for rpt in range(repeat_main):
              for t in range(NT):
                agg_ps = apsum.tile([D, P], f32, tag="agg", name=f"agg{t}")
                nchunks_t = sum(nidx // P for (_b, nidx, _kb, _po) in ops_by_tile[t])
                done = 0
                for (bk, nidx, kb, po) in ops_by_tile[t]:
                    g = gpool.tile([128, nidx], fp16, tag="g", name=f"g{t}_{kb}")
                    bank_lo = bk * BANK
                    bank_hi = min(bank_lo + BANK, N)
                    nc.gpsimd.dma_gather(
                        g[:].rearrange("p (c e) -> p c e", e=D),
                        table[bank_lo:bank_hi, :],
                        idx_sb[:, po : po + nidx // 16],
                        nidx, nidx, D,
                        single_packet=True, queue_num=opi % NQ,
                    )
                    opi += 1
                    for j in range(nidx // P):
                        k = kb + j
                        S = spool.tile([P, P], bf16, tag="S", name=f"S{k}")
                        nc.vector.tensor_scalar(
                            out=S[:], in0=iota_sb[:],
                            scalar1=drel_sb[:, k : k + 1],
                            scalar2=None,
                            op0=mybir.AluOpType.is_equal,
                        )
                        nc.tensor.matmul(
                            out=agg_ps[:],
                            lhsT=g[:, j * D : (j + 1) * D],
                            rhs=S[:],
                            start=(done == 0), stop=(done == nchunks_t - 1),
                        )
                        done += 1

                # stage B: PSUM = ninv*b + agg @ W ; h = relu(ndst * PSUM)
                aggT = apool.tile([D, P], f32, tag="aggT", name=f"aggT{t}")
                nc.vector.tensor_copy(out=aggT[:], in_=agg_ps[:])
                h_ps = hpsum.tile([P, D], f32, tag="hT", name=f"hT{t}")
                nc.tensor.matmul(out=h_ps[:], lhsT=ninv_sb[:, t * P : (t + 1) * P],
                                 rhs=b_row[:], start=True, stop=False)
                nc.tensor.matmul(out=h_ps[:], lhsT=aggT[:], rhs=W_sb[:],
                                 start=False, stop=True)
                hslice = h_sb[:, t * P : (t + 1) * P]
                nc.scalar.activation(
                    out=hslice, in_=h_ps[:],
                    func=mybir.ActivationFunctionType.Relu,
                    scale=ndstT_sb[:, t : t + 1],
                )
                ones_t = ones_last if t == NT - 1 else ones_col
                nc.tensor.matmul(out=ssum_ps[:], lhsT=ones_t[:], rhs=hslice,
                                 start=(t == 0 and rpt == 0),
                                 stop=(t == NT - 1 and rpt == repeat_main - 1),
                                 skip_group_check=True)
                sq = qpool.tile([P, D], f32, tag="sq", name=f"sq{t}")
                nc.scalar.activation(out=sq[:], in_=hslice,
                                     func=mybir.ActivationFunctionType.Square)
                nc.tensor.matmul(out=ssq_ps[:], lhsT=ones_t[:], rhs=sq[:],
                                 start=(t == 0 and rpt == 0),
                                 stop=(t == NT - 1 and rpt == repeat_main - 1),
                                 skip_group_check=True)

            # BN stats -> all-reduce -> A/B rows -> broadcast
            stats = tpool.tile([1, 2 * D], f32)
            nc.vector.tensor_copy(out=stats[:, 0:D], in_=ssum_ps[:])
            nc.vector.tensor_copy(out=stats[:, D : 2 * D], in_=ssq_ps[:])
            nc.sync.dma_start(out=cc_in[:], in_=stats[:])
            if use_collective:
                nc.gpsimd.collective_compute(
                    "AllReduce", mybir.AluOpType.add,
                    replica_groups=[list(range(NCORES))],
                    ins=[cc_in[:]], outs=[cc_out[:]],
                )
            else:
                nc.sync.dma_start(out=cc_out[:], in_=cc_in[:])
            gstats = tpool.tile([1, 2 * D], f32)
            nc.sync.dma_start(out=gstats[:], in_=cc_out[:])

            inv_n = 1.0 / float(N)
            mean = tpool.tile([1, D], f32)
            nc.vector.tensor_scalar_mul(out=mean[:], in0=gstats[:, 0:D], scalar1=inv_n)
            ex2 = tpool.tile([1, D], f32)
            nc.vector.tensor_scalar_mul(out=ex2[:], in0=gstats[:, D : 2 * D], scalar1=inv_n)
            m2 = tpool.tile([1, D], f32)
            nc.vector.tensor_mul(out=m2[:], in0=mean[:], in1=mean[:])
            vare = tpool.tile([1, D], f32)
            nc.vector.tensor_sub(out=vare[:], in0=ex2[:], in1=m2[:])
            nc.vector.tensor_scalar_add(out=vare[:], in0=vare[:], scalar1=BN_EPS)
            rvar = tpool.tile([1, D], f32)
            nc.vector.reciprocal(out=rvar[:], in_=vare[:])
            rstd = tpool.tile([1, D], f32)
            nc.scalar.activation(out=rstd[:], in_=rvar[:],
                                 func=mybir.ActivationFunctionType.Sqrt)
            AB_row = tpool.tile([1, 2 * D], f32)
            nc.vector.tensor_mul(out=AB_row[:, 0:D], in0=g_row[:], in1=rstd[:])
            mA = tpool.tile([1, D], f32)
            nc.vector.tensor_mul(out=mA[:], in0=mean[:], in1=AB_row[:, 0:D])
            nc.vector.tensor_sub(out=AB_row[:, D : 2 * D], in0=bt_row[:], in1=mA[:])
            AB_ps = spsum.tile([P, 2 * D], f32, tag="ABps", name="ABps")
            nc.tensor.matmul(out=AB_ps[:], lhsT=ones_row[:], rhs=AB_row[:],
                             start=True, stop=True, skip_group_check=True)
            AB_sb = tpool.tile([P, 2 * D], f32)
            nc.vector.tensor_copy(out=AB_sb[:], in_=AB_ps[:])

            # apply BN + dropout mask, write out
            for rpt in range(repeat_main):
              for t in range(NT):
                yt = ypool.tile([P, D], f32, tag="yt", name=f"yt{t}")
                nc.vector.tensor_mul(out=yt[:], in0=h_sb[:, t * P : (t + 1) * P],
                                     in1=AB_sb[:, 0:D])
                nc.vector.tensor_add(out=yt[:], in0=yt[:], in1=AB_sb[:, D : 2 * D])
                mk = mpool.tile([P, D], f32, tag="mk", name=f"mk{t}")
                nc.sync.dma_start(out=mk[:], in_=maskv[t * P : (t + 1) * P, :])
                nc.vector.tensor_mul(out=yt[:], in0=yt[:], in1=mk[:])
                nc.sync.dma_start(out=yout[t * P : (t + 1) * P, :], in_=yt[:])

    nc.compile()
    return nc


def kernel(feat, src, dst, W, b, gamma, beta):
    feat = np.ascontiguousarray(np.asarray(feat, np.float32))
    src = np.asarray(src).astype(np.int64)
    dst = np.asarray(dst).astype(np.int64)
    N, D = feat.shape
    assert D == 128 and N % NCORES == 0

    meta, idx_all, rel_all, norm_src, norm_dst = _host_prep(N, D, src, dst)
    rpc, NT = meta["rpc"], meta["NT"]
    rpad = NT * P

    table16 = (feat * norm_src[:, None]).astype(np.float16)

    import jax
    with jax.default_device(jax.devices("cpu")[0]):
        keep = jax.random.bernoulli(jax.random.key(42), 0.5, (N, D))
        maskval = (np.asarray(keep).astype(np.float32)) * 2.0

    nc = _build_program(meta)

    in_maps = []
    for c in range(NCORES):
        nd = np.ones(rpad, np.float32)
        nd[:rpc] = norm_dst[c * rpc : (c + 1) * rpc]
        ndT = nd.reshape(NT, P).T.copy()           # [P, NT] col t = tile t
        ninv_a = (1.0 / nd).reshape(NT, P).copy()  # [NT, P]
        mrows = np.zeros((rpad, D), np.float32)
        mrows[:rpc] = maskval[c * rpc : (c + 1) * rpc]
        in_maps.append({
            "table": table16,
            "idxs": np.ascontiguousarray(idx_all[c]),
            "drel": np.ascontiguousarray(rel_all[c]),
            "ndstT": np.ascontiguousarray(ndT),
            "ninv": np.ascontiguousarray(ninv_a),
            "maskv": mrows,
            "W": np.ascontiguousarray(np.asarray(W, np.float32)),
            "b": np.ascontiguousarray(np.asarray(b, np.float32)),
            "gamma": np.ascontiguousarray(np.asarray(gamma, np.float32)),
            "beta": np.ascontiguousarray(np.asarray(beta, np.float32)),
        })

    res = run_bass_kernel_spmd(nc, in_maps, list(range(NCORES)))
    out = np.concatenate([res.results[c]["y"][:rpc] for c in range(NCORES)], axis=0)
    return out


# revision 11
# speedup vs baseline: 1.5144x; 1.5144x over previous
"""GCN layer (GraphConv + BN + dropout) as a Trainium2 Bass kernel, SPMD over 8 NeuronCores.

v2 design:
- dst-node sharding across 8 cores; gather table = feat * norm_src (folded on host,
  fp16) replicated per core; edges partitioned by (dst-tile, src-bank), padded to
  128-edge chunks with counts uniform across cores (max) so one NEFF runs SPMD.
- aggregation: dma_gather (4 SWDGE queues) + per-chunk 0/1 one-hot S (bf16, exact)
  matmul-accumulated into PSUM: aggT[f, d] += G^T S.
- stage B per tile: PSUM = invnorm_dst*b (K=1 matmul) + agg @ W, then
  h = relu(norm_dst * PSUM) via activation per-partition scale. BN stats via
  ones-matmul accumulation, all-reduced across cores; apply + dropout mask on DVE.
"""
import sys

sys.path.insert(0, "/opt/trn_rl_repo")

import numpy as np
import ml_dtypes

import concourse.bass as bass
import concourse.bacc as bacc
import concourse.mybir as mybir
import concourse.tile as tile
from concourse.bass_utils import run_bass_kernel_spmd

NCORES = 8
P = 128
BANK = 32768          # rows addressable by int16 gather index
OPC = 8               # max chunks (of 128 rows) per dma_gather op
NQ = 4                # SWDGE queues to rotate gather ops over
BN_EPS = 1e-5


def _host_prep(N, D, src, dst):
    """Partition edges by (core, dst-tile, src-bank); pad to 128-edge chunks with
    counts uniform across cores (max), producing per-core streams."""
    rpc = N // NCORES                      # rows per core
    NT = (rpc + P - 1) // P                # dst tiles per core
    NB = (N + BANK - 1) // BANK            # src banks

    deg_out = np.bincount(src, minlength=N).astype(np.float64)
    deg_in = np.bincount(dst, minlength=N).astype(np.float64)
    norm_src = (np.where(deg_out > 0, deg_out, 1.0) ** -0.5).astype(np.float32)
    norm_dst = (np.where(deg_in > 0, deg_in, 1.0) ** -0.5).astype(np.float32)

    core_of = dst // rpc
    ldst = dst - core_of * rpc
    tile_of = ldst // P
    rel_of = (ldst % P).astype(np.float32)
    bank_of = src >> 15
    lsrc = (src & (BANK - 1)).astype(np.int16)
    seg_of = tile_of * NB + bank_of        # segment id within a core

    NSEG = NT * NB
    per_core = []
    L = np.zeros((NCORES, NSEG), np.int64)
    for c in range(NCORES):
        m = core_of == c
        order = np.argsort(seg_of[m], kind="stable")
        idx = np.nonzero(m)[0][order]
        per_core.append((lsrc[idx], rel_of[idx]))
        L[c] = np.bincount(seg_of[m], minlength=NSEG)

    n_chunks = (L.max(axis=0) + P - 1) // P        # [NSEG], uniform over cores
    seg_pad = n_chunks * P
    total_edges = int(seg_pad.sum())
    TC = total_edges // P                          # total chunks per core

    ops = []  # (bank, nidx, chunk_base)
    k = 0
    for s in range(NSEG):
        nc_s = int(n_chunks[s])
        if nc_s == 0:
            continue
        b = s % NB
        done = 0
        while done < nc_s:
            take = min(OPC, nc_s - done)
            ops.append((b, take * P, k))
            k += take
            done += take
    assert k == TC

    idx_w = total_edges // 16
    idx_all = np.zeros((NCORES, 128, idx_w), np.int16)
    rel_all = np.full((NCORES, P, TC), -1.0, np.float32)
    seg_off = np.zeros(NSEG + 1, np.int64)
    np.cumsum(seg_pad, out=seg_off[1:])
    for c in range(NCORES):
        ls, rel = per_core[c]
        src_pad = np.zeros(total_edges, np.int16)
        rel_pad = np.full(total_edges, -1.0, np.float32)
        off = np.cumsum(np.concatenate([[0], L[c]]))[:-1]
        for s in range(NSEG):
            n = int(L[c, s])
            if n == 0:
                continue
            dstart = int(seg_off[s])
            src_pad[dstart : dstart + n] = ls[off[s] : off[s] + n]
            rel_pad[dstart : dstart + n] = rel[off[s] : off[s] + n]
        rel_all[c] = rel_pad.reshape(TC, P).T
        pos = 0
        for (_b, nidx, kb) in ops:
            seg = src_pad[kb * P : kb * P + nidx]
            blk = seg.reshape(-1, 16).T          # [16, nidx/16]
            idx_all[c, :, pos : pos + nidx // 16] = np.tile(blk, (8, 1))
            pos += nidx // 16
        assert pos == idx_w

    meta = dict(N=N, D=D, rpc=rpc, NT=NT, NB=NB, TC=TC, idx_w=idx_w,
                n_chunks=n_chunks, ops=ops, total_edges=total_edges)
    return meta, idx_all, rel_all, norm_src, norm_dst


def _build_program(meta, repeat_main=1, use_collective=True):
    N, D = meta["N"], meta["D"]
    NT, NB, TC = meta["NT"], meta["NB"], meta["TC"]
    idx_w, ops, n_chunks = meta["idx_w"], meta["ops"], meta["n_chunks"]
    rpad = NT * P
    nvalid_last = meta["rpc"] - (NT - 1) * P       # valid rows in last tile
    f32 = mybir.dt.float32
    bf16 = mybir.dt.bfloat16
    fp16 = mybir.dt.float16

    nc = bacc.Bacc("TRN2", num_devices=NCORES, num_swdge_queues=NQ)
    table = nc.declare_dram_parameter("table", [N, D], fp16, isOutput=False)
    idxs = nc.declare_dram_parameter("idxs", [128, idx_w], mybir.dt.int16, isOutput=False)
    drel = nc.declare_dram_parameter("drel", [P, TC], f32, isOutput=False)
    ndstT = nc.declare_dram_parameter("ndstT", [P, NT], f32, isOutput=False)
    ninv = nc.declare_dram_parameter("ninv", [NT, P], f32, isOutput=False)
    maskv = nc.declare_dram_parameter("maskv", [rpad, D], f32, isOutput=False)
    Wp = nc.declare_dram_parameter("W", [D, D], f32, isOutput=False)
    bp = nc.declare_dram_parameter("b", [D], f32, isOutput=False)
    gp = nc.declare_dram_parameter("gamma", [D], f32, isOutput=False)
    btp = nc.declare_dram_parameter("beta", [D], f32, isOutput=False)
    yout = nc.declare_dram_parameter("y", [rpad, D], f32, isOutput=True)

    cc_in = nc.dram_tensor("cc_in", [1, 2 * D], f32)
    cc_out = nc.dram_tensor("cc_out", [1, 2 * D], f32, addr_space="Shared")

    iota_np = np.tile(np.arange(P).astype(ml_dtypes.bfloat16), (P, 1))
    iota_dram = nc.inline_tensor(iota_np, name="iota_const")
    ones_last_np = np.zeros((P, 1), np.float32)
    ones_last_np[:nvalid_last] = 1.0
    ones_last_dram = nc.inline_tensor(ones_last_np, name="ones_last_const")

    ops_by_tile = [[] for _ in range(NT)]
    chunk_tile = np.repeat(np.arange(NT * NB) // NB, n_chunks)
    pos = 0
    for (b, nidx, kb) in ops:
        ops_by_tile[int(chunk_tile[kb])].append((b, nidx, kb, pos))
        pos += nidx // 16

    with tile.TileContext(nc) as tc:
        with tc.tile_pool(name="consts", bufs=1) as cpool, \
             tc.tile_pool(name="streams", bufs=1) as stpool, \
             tc.tile_pool(name="hbuf", bufs=1) as hpool, \
             tc.tile_pool(name="gat", bufs=10) as gpool, \
             tc.tile_pool(name="sel", bufs=24) as spool, \
             tc.tile_pool(name="aggsb", bufs=3) as apool, \
             tc.tile_pool(name="sq", bufs=2) as qpool, \
             tc.tile_pool(name="ybuf", bufs=4) as ypool, \
             tc.tile_pool(name="msk", bufs=4) as mpool, \
             tc.tile_pool(name="stat", bufs=1) as tpool, \
             tc.tile_pool(name="apsum", bufs=3, space="PSUM") as apsum, \
             tc.tile_pool(name="hpsum", bufs=2, space="PSUM") as hpsum, \
             tc.tile_pool(name="spsum", bufs=1, space="PSUM") as spsum:

            iota_sb = cpool.tile([P, P], bf16)
            nc.sync.dma_start(out=iota_sb[:], in_=iota_dram[:])
            W_sb = cpool.tile([D, D], f32)
            nc.sync.dma_start(out=W_sb[:], in_=Wp[:])
            b_row = cpool.tile([1, D], f32)
            nc.sync.dma_start(out=b_row[:], in_=bp[None, :])
            g_row = cpool.tile([1, D], f32)
            nc.sync.dma_start(out=g_row[:], in_=gp[None, :])
            bt_row = cpool.tile([1, D], f32)
            nc.sync.dma_start(out=bt_row[:], in_=btp[None, :])
            ones_col = cpool.tile([P, 1], f32)
            nc.vector.memset(ones_col[:], 1.0)
            ones_last = cpool.tile([P, 1], f32)
            nc.sync.dma_start(out=ones_last[:], in_=ones_last_dram[:])
            ones_row = cpool.tile([1, P], f32)
            nc.vector.memset(ones_row[:], 1.0)

            idx_sb = stpool.tile([128, idx_w], mybir.dt.int16)
            nc.sync.dma_start(out=idx_sb[:], in_=idxs[:])
            drel_sb = stpool.tile([P, TC], f32)
            nc.sync.dma_start(out=drel_sb[:], in_=drel[:])
            ndstT_sb = stpool.tile([P, NT], f32)
            nc.sync.dma_start(out=ndstT_sb[:], in_=ndstT[:])
            ninv_sb = stpool.tile([1, NT * P], f32)
            nc.sync.dma_start(out=ninv_sb[:], in_=ninv[:].rearrange("t p -> (t p)")[None, :])

            h_sb = hpool.tile([P, rpad], f32)

            ssum_ps = spsum.tile([1, D], f32, tag="ssum", name="ssum")
            ssq_ps = spsum.tile([1, D], f32, tag="ssq", name="ssq")

            opi = 0
            for rpt in range(repeat_main):
              for t in range(NT):
                agg_ps = apsum.tile([D, P], f32, tag="agg", name=f"agg{t}")
                nchunks_t = sum(nidx // P for (_b, nidx, _kb, _po) in ops_by_tile[t])
                done = 0
                for (bk, nidx, kb, po) in ops_by_tile[t]:
                    g = gpool.tile([128, nidx], fp16, tag="g", name=f"g{t}_{kb}")
                    bank_lo = bk * BANK
                    bank_hi = min(bank_lo + BANK, N)
                    nc.gpsimd.dma_gather(
                        g[:].rearrange("p (c e) -> p c e", e=D),
                        table[bank_lo:bank_hi, :],
                        idx_sb[:, po : po + nidx // 16],
                        nidx, nidx, D,
                        single_packet=True, queue_num=opi % NQ,
                    )
                    opi += 1
                    for j in range(nidx // P):
                        k = kb + j
                        S = spool.tile([P, P], bf16, tag="S", name=f"S{k}")
                        nc.vector.tensor_scalar(
                            out=S[:], in0=iota_sb[:],
                            scalar1=drel_sb[:, k : k + 1],
                            scalar2=None,
                            op0=mybir.AluOpType.is_equal,
                        )
                        nc.tensor.matmul(
                            out=agg_ps[:],
                            lhsT=g[:, j * D : (j + 1) * D],
                            rhs=S[:],
                            start=(done == 0), stop=(done == nchunks_t - 1),
                        )
                        done += 1

                # stage B: PSUM = ninv*b + agg @ W ; h = relu(ndst * PSUM)
                aggT = apool.tile([D, P], f32, tag="aggT", name=f"aggT{t}")
                nc.scalar.copy(out=aggT[:], in_=agg_ps[:])
                h_ps = hpsum.tile([P, D], f32, tag="hT", name=f"hT{t}")
                nc.tensor.matmul(out=h_ps[:], lhsT=ninv_sb[:, t * P : (t + 1) * P],
                                 rhs=b_row[:], start=True, stop=False)
                nc.tensor.matmul(out=h_ps[:], lhsT=aggT[:], rhs=W_sb[:],
                                 start=False, stop=True)
                hslice = h_sb[:, t * P : (t + 1) * P]
                nc.scalar.activation(
                    out=hslice, in_=h_ps[:],
                    func=mybir.ActivationFunctionType.Relu,
                    scale=ndstT_sb[:, t : t + 1],
                )
                ones_t = ones_last if t == NT - 1 else ones_col
                nc.tensor.matmul(out=ssum_ps[:], lhsT=ones_t[:], rhs=hslice,
                                 start=(t == 0 and rpt == 0),
                                 stop=(t == NT - 1 and rpt == repeat_main - 1),
                                 skip_group_check=True)
                sq = qpool.tile([P, D], f32, tag="sq", name=f"sq{t}")
                nc.scalar.activation(out=sq[:], in_=hslice,
                                     func=mybir.ActivationFunctionType.Square)
                nc.tensor.matmul(out=ssq_ps[:], lhsT=ones_t[:], rhs=sq[:],
                                 start=(t == 0 and rpt == 0),
                                 stop=(t == NT - 1 and rpt == repeat_main - 1),
                                 skip_group_check=True)

            # BN stats -> all-reduce -> A/B rows -> broadcast
            stats = tpool.tile([1, 2 * D], f32)
            nc.vector.tensor_copy(out=stats[:, 0:D], in_=ssum_ps[:])
            nc.vector.tensor_copy(out=stats[:, D : 2 * D], in_=ssq_ps[:])
            nc.sync.dma_start(out=cc_in[:], in_=stats[:])
            if use_collective:
                nc.gpsimd.collective_compute(
                    "AllReduce", mybir.AluOpType.add,
                    replica_groups=[list(range(NCORES))],
                    ins=[cc_in[:]], outs=[cc_out[:]],
                )
            else:
                nc.sync.dma_start(out=cc_out[:], in_=cc_in[:])
            gstats = tpool.tile([1, 2 * D], f32)
            nc.sync.dma_start(out=gstats[:], in_=cc_out[:])

            inv_n = 1.0 / float(N)
            mean = tpool.tile([1, D], f32)
            nc.vector.tensor_scalar_mul(out=mean[:], in0=gstats[:, 0:D], scalar1=inv_n)
            ex2 = tpool.tile([1, D], f32)
            nc.vector.tensor_scalar_mul(out=ex2[:], in0=gstats[:, D : 2 * D], scalar1=inv_n)
            m2 = tpool.tile([1, D], f32)
            nc.vector.tensor_mul(out=m2[:], in0=mean[:], in1=mean[:])
            vare = tpool.tile([1, D], f32)
            nc.vector.tensor_sub(out=vare[:], in0=ex2[:], in1=m2[:])
            nc.vector.tensor_scalar_add(out=vare[:], in0=vare[:], scalar1=BN_EPS)
            rvar = tpool.tile([1, D], f32)
            nc.vector.reciprocal(out=rvar[:], in_=vare[:])
            rstd = tpool.tile([1, D], f32)
            nc.scalar.activation(out=rstd[:], in_=rvar[:],
                                 func=mybir.ActivationFunctionType.Sqrt)
            AB_row = tpool.tile([1, 2 * D], f32)
            nc.vector.tensor_mul(out=AB_row[:, 0:D], in0=g_row[:], in1=rstd[:])
            mA = tpool.tile([1, D], f32)
            nc.vector.tensor_mul(out=mA[:], in0=mean[:], in1=AB_row[:, 0:D])
            nc.vector.tensor_sub(out=AB_row[:, D : 2 * D], in0=bt_row[:], in1=mA[:])
            AB_ps = spsum.tile([P, 2 * D], f32, tag="ABps", name="ABps")
            nc.tensor.matmul(out=AB_ps[:], lhsT=ones_row[:], rhs=AB_row[:],
                             start=True, stop=True, skip_group_check=True)
            AB_sb = tpool.tile([P, 2 * D], f32)
            nc.vector.tensor_copy(out=AB_sb[:], in_=AB_ps[:])

            # apply BN + dropout mask, write out
            for rpt in range(repeat_main):
              for t in range(NT):
                yt = ypool.tile([P, D], f32, tag="yt", name=f"yt{t}")
                nc.vector.tensor_mul(out=yt[:], in0=h_sb[:, t * P : (t + 1) * P],
                                     in1=AB_sb[:, 0:D])
                nc.vector.tensor_add(out=yt[:], in0=yt[:], in1=AB_sb[:, D : 2 * D])
                mk = mpool.tile([P, D], f32, tag="mk", name=f"mk{t}")
                nc.sync.dma_start(out=mk[:], in_=maskv[t * P : (t + 1) * P, :])
                nc.vector.tensor_mul(out=yt[:], in0=yt[:], in1=mk[:])
                nc.sync.dma_start(out=yout[t * P : (t + 1) * P, :], in_=yt[:])

    nc.compile()
    return nc


def kernel(feat, src, dst, W, b, gamma, beta):
    feat = np.ascontiguousarray(np.asarray(feat, np.float32))
    src = np.asarray(src).astype(np.int64)
    dst = np.asarray(dst).astype(np.int64)
    N, D = feat.shape
    assert D == 128 and N % NCORES == 0

    meta, idx_all, rel_all, norm_src, norm_dst = _host_prep(N, D, src, dst)
    rpc, NT = meta["rpc"], meta["NT"]
    rpad = NT * P

    table16 = (feat * norm_src[:, None]).astype(np.float16)

    import jax
    with jax.default_device(jax.devices("cpu")[0]):
        keep = jax.random.bernoulli(jax.random.key(42), 0.5, (N, D))
        maskval = (np.asarray(keep).astype(np.float32)) * 2.0

    nc = _build_program(meta)

    in_maps = []
    for c in range(NCORES):
        nd = np.ones(rpad, np.float32)
        nd[:rpc] = norm_dst[c * rpc : (c + 1) * rpc]
        ndT = nd.reshape(NT, P).T.copy()           # [P, NT] col t = tile t
        ninv_a = (1.0 / nd).reshape(NT, P).copy()  # [NT, P]
        mrows = np.zeros((rpad, D), np.float32)
        mrows[:rpc] = maskval[c * rpc : (c + 1) * rpc]
        in_maps.append({
            "table": table16,
            "idxs": np.ascontiguousarray(idx_all[c]),
            "drel": np.ascontiguousarray(rel_all[c]),
            "ndstT": np.ascontiguousarray(ndT),
            "ninv": np.ascontiguousarray(ninv_a),
            "maskv": mrows,
            "W": np.ascontiguousarray(np.asarray(W, np.float32)),
            "b": np.ascontiguousarray(np.asarray(b, np.float32)),
            "gamma": np.ascontiguousarray(np.asarray(gamma, np.float32)),
            "beta": np.ascontiguousarray(np.asarray(beta, np.float32)),
        })

    res = run_bass_kernel_spmd(nc, in_maps, list(range(NCORES)))
    out = np.concatenate([res.results[c]["y"][:rpc] for c in range(NCORES)], axis=0)
    return out


# revision 12
# speedup vs baseline: 1.5836x; 1.0457x over previous
"""GCN layer (GraphConv + BN + dropout) as a Trainium2 Bass kernel, SPMD over 8 NeuronCores.

v2 design:
- dst-node sharding across 8 cores; gather table = feat * norm_src (folded on host,
  fp16) replicated per core; edges partitioned by (dst-tile, src-bank), padded to
  128-edge chunks with counts uniform across cores (max) so one NEFF runs SPMD.
- aggregation: dma_gather (4 SWDGE queues) + per-chunk 0/1 one-hot S (bf16, exact)
  matmul-accumulated into PSUM: aggT[f, d] += G^T S.
- stage B per tile: PSUM = invnorm_dst*b (K=1 matmul) + agg @ W, then
  h = relu(norm_dst * PSUM) via activation per-partition scale. BN stats via
  ones-matmul accumulation, all-reduced across cores; apply + dropout mask on DVE.
"""
import sys

sys.path.insert(0, "/opt/trn_rl_repo")

import numpy as np
import ml_dtypes

import concourse.bass as bass
import concourse.bacc as bacc
import concourse.mybir as mybir
import concourse.tile as tile
from concourse.bass_utils import run_bass_kernel_spmd

NCORES = 8
P = 128
BANK = 32768          # rows addressable by int16 gather index
OPC = 16              # max chunks (of 128 rows) per dma_gather op
NQ = 4                # SWDGE queues to rotate gather ops over
BN_EPS = 1e-5


def _host_prep(N, D, src, dst):
    """Partition edges by (core, dst-tile, src-bank); pad to 128-edge chunks with
    counts uniform across cores (max), producing per-core streams."""
    rpc = N // NCORES                      # rows per core
    NT = (rpc + P - 1) // P                # dst tiles per core
    NB = (N + BANK - 1) // BANK            # src banks

    deg_out = np.bincount(src, minlength=N).astype(np.float64)
    deg_in = np.bincount(dst, minlength=N).astype(np.float64)
    norm_src = (np.where(deg_out > 0, deg_out, 1.0) ** -0.5).astype(np.float32)
    norm_dst = (np.where(deg_in > 0, deg_in, 1.0) ** -0.5).astype(np.float32)

    core_of = dst // rpc
    ldst = dst - core_of * rpc
    tile_of = ldst // P
    rel_of = (ldst % P).astype(np.float32)
    bank_of = src >> 15
    lsrc = (src & (BANK - 1)).astype(np.int16)
    seg_of = tile_of * NB + bank_of        # segment id within a core

    NSEG = NT * NB
    per_core = []
    L = np.zeros((NCORES, NSEG), np.int64)
    for c in range(NCORES):
        m = core_of == c
        order = np.argsort(seg_of[m], kind="stable")
        idx = np.nonzero(m)[0][order]
        per_core.append((lsrc[idx], rel_of[idx]))
        L[c] = np.bincount(seg_of[m], minlength=NSEG)

    n_chunks = (L.max(axis=0) + P - 1) // P        # [NSEG], uniform over cores
    seg_pad = n_chunks * P
    total_edges = int(seg_pad.sum())
    TC = total_edges // P                          # total chunks per core

    ops = []  # (bank, nidx, chunk_base)
    k = 0
    for s in range(NSEG):
        nc_s = int(n_chunks[s])
        if nc_s == 0:
            continue
        b = s % NB
        done = 0
        while done < nc_s:
            take = min(OPC, nc_s - done)
            ops.append((b, take * P, k))
            k += take
            done += take
    assert k == TC

    idx_w = total_edges // 16
    idx_all = np.zeros((NCORES, 128, idx_w), np.int16)
    rel_all = np.full((NCORES, P, TC), -1.0, np.float32)
    seg_off = np.zeros(NSEG + 1, np.int64)
    np.cumsum(seg_pad, out=seg_off[1:])
    for c in range(NCORES):
        ls, rel = per_core[c]
        src_pad = np.zeros(total_edges, np.int16)
        rel_pad = np.full(total_edges, -1.0, np.float32)
        off = np.cumsum(np.concatenate([[0], L[c]]))[:-1]
        for s in range(NSEG):
            n = int(L[c, s])
            if n == 0:
                continue
            dstart = int(seg_off[s])
            src_pad[dstart : dstart + n] = ls[off[s] : off[s] + n]
            rel_pad[dstart : dstart + n] = rel[off[s] : off[s] + n]
        rel_all[c] = rel_pad.reshape(TC, P).T
        pos = 0
        for (_b, nidx, kb) in ops:
            seg = src_pad[kb * P : kb * P + nidx]
            blk = seg.reshape(-1, 16).T          # [16, nidx/16]
            idx_all[c, :, pos : pos + nidx // 16] = np.tile(blk, (8, 1))
            pos += nidx // 16
        assert pos == idx_w

    meta = dict(N=N, D=D, rpc=rpc, NT=NT, NB=NB, TC=TC, idx_w=idx_w,
                n_chunks=n_chunks, ops=ops, total_edges=total_edges)
    return meta, idx_all, rel_all, norm_src, norm_dst


def _build_program(meta, repeat_main=1, use_collective=True):
    N, D = meta["N"], meta["D"]
    NT, NB, TC = meta["NT"], meta["NB"], meta["TC"]
    idx_w, ops, n_chunks = meta["idx_w"], meta["ops"], meta["n_chunks"]
    rpad = NT * P
    nvalid_last = meta["rpc"] - (NT - 1) * P       # valid rows in last tile
    f32 = mybir.dt.float32
    bf16 = mybir.dt.bfloat16
    fp16 = mybir.dt.float16

    nc = bacc.Bacc("TRN2", num_devices=NCORES, num_swdge_queues=NQ)
    table = nc.declare_dram_parameter("table", [N, D], fp16, isOutput=False)
    idxs = nc.declare_dram_parameter("idxs", [128, idx_w], mybir.dt.int16, isOutput=False)
    drel = nc.declare_dram_parameter("drel", [P, TC], f32, isOutput=False)
    ndstT = nc.declare_dram_parameter("ndstT", [P, NT], f32, isOutput=False)
    ninv = nc.declare_dram_parameter("ninv", [NT, P], f32, isOutput=False)
    maskv = nc.declare_dram_parameter("maskv", [rpad, D], f32, isOutput=False)
    Wp = nc.declare_dram_parameter("W", [D, D], f32, isOutput=False)
    bp = nc.declare_dram_parameter("b", [D], f32, isOutput=False)
    gp = nc.declare_dram_parameter("gamma", [D], f32, isOutput=False)
    btp = nc.declare_dram_parameter("beta", [D], f32, isOutput=False)
    yout = nc.declare_dram_parameter("y", [rpad, D], f32, isOutput=True)

    cc_in = nc.dram_tensor("cc_in", [1, 2 * D], f32)
    cc_out = nc.dram_tensor("cc_out", [1, 2 * D], f32, addr_space="Shared")

    iota_np = np.tile(np.arange(P).astype(ml_dtypes.bfloat16), (P, 1))
    iota_dram = nc.inline_tensor(iota_np, name="iota_const")
    ones_last_np = np.zeros((P, 1), np.float32)
    ones_last_np[:nvalid_last] = 1.0
    ones_last_dram = nc.inline_tensor(ones_last_np, name="ones_last_const")

    ops_by_tile = [[] for _ in range(NT)]
    chunk_tile = np.repeat(np.arange(NT * NB) // NB, n_chunks)
    pos = 0
    for (b, nidx, kb) in ops:
        ops_by_tile[int(chunk_tile[kb])].append((b, nidx, kb, pos))
        pos += nidx // 16

    with tile.TileContext(nc) as tc:
        with tc.tile_pool(name="consts", bufs=1) as cpool, \
             tc.tile_pool(name="streams", bufs=1) as stpool, \
             tc.tile_pool(name="hbuf", bufs=1) as hpool, \
             tc.tile_pool(name="gat", bufs=10) as gpool, \
             tc.tile_pool(name="sel", bufs=24) as spool, \
             tc.tile_pool(name="aggsb", bufs=3) as apool, \
             tc.tile_pool(name="sq", bufs=2) as qpool, \
             tc.tile_pool(name="ybuf", bufs=4) as ypool, \
             tc.tile_pool(name="msk", bufs=4) as mpool, \
             tc.tile_pool(name="stat", bufs=1) as tpool, \
             tc.tile_pool(name="apsum", bufs=3, space="PSUM") as apsum, \
             tc.tile_pool(name="hpsum", bufs=2, space="PSUM") as hpsum, \
             tc.tile_pool(name="spsum", bufs=1, space="PSUM") as spsum:

            iota_sb = cpool.tile([P, P], bf16)
            nc.sync.dma_start(out=iota_sb[:], in_=iota_dram[:])
            W_sb = cpool.tile([D, D], f32)
            nc.sync.dma_start(out=W_sb[:], in_=Wp[:])
            b_row = cpool.tile([1, D], f32)
            nc.sync.dma_start(out=b_row[:], in_=bp[None, :])
            g_row = cpool.tile([1, D], f32)
            nc.sync.dma_start(out=g_row[:], in_=gp[None, :])
            bt_row = cpool.tile([1, D], f32)
            nc.sync.dma_start(out=bt_row[:], in_=btp[None, :])
            ones_col = cpool.tile([P, 1], f32)
            nc.vector.memset(ones_col[:], 1.0)
            ones_last = cpool.tile([P, 1], f32)
            nc.sync.dma_start(out=ones_last[:], in_=ones_last_dram[:])
            ones_row = cpool.tile([1, P], f32)
            nc.vector.memset(ones_row[:], 1.0)

            idx_sb = stpool.tile([128, idx_w], mybir.dt.int16)
            nc.sync.dma_start(out=idx_sb[:], in_=idxs[:])
            drel_sb = stpool.tile([P, TC], f32)
            nc.sync.dma_start(out=drel_sb[:], in_=drel[:])
            ndstT_sb = stpool.tile([P, NT], f32)
            nc.sync.dma_start(out=ndstT_sb[:], in_=ndstT[:])
            ninv_sb = stpool.tile([1, NT * P], f32)
            nc.sync.dma_start(out=ninv_sb[:], in_=ninv[:].rearrange("t p -> (t p)")[None, :])

            h_sb = hpool.tile([P, rpad], f32)

            ssum_ps = spsum.tile([1, D], f32, tag="ssum", name="ssum")
            ssq_ps = spsum.tile([1, D], f32, tag="ssq", name="ssq")

            opi = 0
            for rpt in range(repeat_main):
              for t in range(NT):
                agg_ps = apsum.tile([D, P], f32, tag="agg", name=f"agg{t}")
                nchunks_t = sum(nidx // P for (_b, nidx, _kb, _po) in ops_by_tile[t])
                done = 0
                for (bk, nidx, kb, po) in ops_by_tile[t]:
                    g = gpool.tile([128, nidx], fp16, tag="g", name=f"g{t}_{kb}")
                    bank_lo = bk * BANK
                    bank_hi = min(bank_lo + BANK, N)
                    nc.gpsimd.dma_gather(
                        g[:].rearrange("p (c e) -> p c e", e=D),
                        table[bank_lo:bank_hi, :],
                        idx_sb[:, po : po + nidx // 16],
                        nidx, nidx, D,
                        single_packet=(nidx <= 1024), queue_num=opi % NQ,
                    )
                    opi += 1
                    for j in range(nidx // P):
                        k = kb + j
                        S = spool.tile([P, P], bf16, tag="S", name=f"S{k}")
                        nc.vector.tensor_scalar(
                            out=S[:], in0=iota_sb[:],
                            scalar1=drel_sb[:, k : k + 1],
                            scalar2=None,
                            op0=mybir.AluOpType.is_equal,
                        )
                        nc.tensor.matmul(
                            out=agg_ps[:],
                            lhsT=g[:, j * D : (j + 1) * D],
                            rhs=S[:],
                            start=(done == 0), stop=(done == nchunks_t - 1),
                        )
                        done += 1

                # stage B: PSUM = ninv*b + agg @ W ; h = relu(ndst * PSUM)
                aggT = apool.tile([D, P], f32, tag="aggT", name=f"aggT{t}")
                nc.scalar.copy(out=aggT[:], in_=agg_ps[:])
                h_ps = hpsum.tile([P, D], f32, tag="hT", name=f"hT{t}")
                nc.tensor.matmul(out=h_ps[:], lhsT=ninv_sb[:, t * P : (t + 1) * P],
                                 rhs=b_row[:], start=True, stop=False)
                nc.tensor.matmul(out=h_ps[:], lhsT=aggT[:], rhs=W_sb[:],
                                 start=False, stop=True)
                hslice = h_sb[:, t * P : (t + 1) * P]
                nc.scalar.activation(
                    out=hslice, in_=h_ps[:],
                    func=mybir.ActivationFunctionType.Relu,
                    scale=ndstT_sb[:, t : t + 1],
                )
                ones_t = ones_last if t == NT - 1 else ones_col
                nc.tensor.matmul(out=ssum_ps[:], lhsT=ones_t[:], rhs=hslice,
                                 start=(t == 0 and rpt == 0),
                                 stop=(t == NT - 1 and rpt == repeat_main - 1),
                                 skip_group_check=True)
                sq = qpool.tile([P, D], f32, tag="sq", name=f"sq{t}")
                nc.scalar.activation(out=sq[:], in_=hslice,
                                     func=mybir.ActivationFunctionType.Square)
                nc.tensor.matmul(out=ssq_ps[:], lhsT=ones_t[:], rhs=sq[:],
                                 start=(t == 0 and rpt == 0),
                                 stop=(t == NT - 1 and rpt == repeat_main - 1),
                                 skip_group_check=True)

            # BN stats -> all-reduce -> A/B rows -> broadcast
            stats = tpool.tile([1, 2 * D], f32)
            nc.vector.tensor_copy(out=stats[:, 0:D], in_=ssum_ps[:])
            nc.vector.tensor_copy(out=stats[:, D : 2 * D], in_=ssq_ps[:])
            nc.sync.dma_start(out=cc_in[:], in_=stats[:])
            if use_collective:
                nc.gpsimd.collective_compute(
                    "AllReduce", mybir.AluOpType.add,
                    replica_groups=[list(range(NCORES))],
                    ins=[cc_in[:]], outs=[cc_out[:]],
                )
            else:
                nc.sync.dma_start(out=cc_out[:], in_=cc_in[:])
            gstats = tpool.tile([1, 2 * D], f32)
            nc.sync.dma_start(out=gstats[:], in_=cc_out[:])

            inv_n = 1.0 / float(N)
            mean = tpool.tile([1, D], f32)
            nc.vector.tensor_scalar_mul(out=mean[:], in0=gstats[:, 0:D], scalar1=inv_n)
            ex2 = tpool.tile([1, D], f32)
            nc.vector.tensor_scalar_mul(out=ex2[:], in0=gstats[:, D : 2 * D], scalar1=inv_n)
            m2 = tpool.tile([1, D], f32)
            nc.vector.tensor_mul(out=m2[:], in0=mean[:], in1=mean[:])
            vare = tpool.tile([1, D], f32)
            nc.vector.tensor_sub(out=vare[:], in0=ex2[:], in1=m2[:])
            nc.vector.tensor_scalar_add(out=vare[:], in0=vare[:], scalar1=BN_EPS)
            rvar = tpool.tile([1, D], f32)
            nc.vector.reciprocal(out=rvar[:], in_=vare[:])
            rstd = tpool.tile([1, D], f32)
            nc.scalar.activation(out=rstd[:], in_=rvar[:],
                                 func=mybir.ActivationFunctionType.Sqrt)
            AB_row = tpool.tile([1, 2 * D], f32)
            nc.vector.tensor_mul(out=AB_row[:, 0:D], in0=g_row[:], in1=rstd[:])
            mA = tpool.tile([1, D], f32)
            nc.vector.tensor_mul(out=mA[:], in0=mean[:], in1=AB_row[:, 0:D])
            nc.vector.tensor_sub(out=AB_row[:, D : 2 * D], in0=bt_row[:], in1=mA[:])
            AB_ps = spsum.tile([P, 2 * D], f32, tag="ABps", name="ABps")
            nc.tensor.matmul(out=AB_ps[:], lhsT=ones_row[:], rhs=AB_row[:],
                             start=True, stop=True, skip_group_check=True)
            AB_sb = tpool.tile([P, 2 * D], f32)
            nc.vector.tensor_copy(out=AB_sb[:], in_=AB_ps[:])

            # apply BN + dropout mask, write out
            for rpt in range(repeat_main):
              for t in range(NT):
                yt = ypool.tile([P, D], f32, tag="yt", name=f"yt{t}")
                nc.vector.tensor_mul(out=yt[:], in0=h_sb[:, t * P : (t + 1) * P],
                                     in1=AB_sb[:, 0:D])
                nc.vector.tensor_add(out=yt[:], in0=yt[:], in1=AB_sb[:, D : 2 * D])
                mk = mpool.tile([P, D], f32, tag="mk", name=f"mk{t}")
                nc.sync.dma_start(out=mk[:], in_=maskv[t * P : (t + 1) * P, :])
                nc.vector.tensor_mul(out=yt[:], in0=yt[:], in1=mk[:])
                nc.sync.dma_start(out=yout[t * P : (t + 1) * P, :], in_=yt[:])

    nc.compile()
    return nc


def kernel(feat, src, dst, W, b, gamma, beta):
    feat = np.ascontiguousarray(np.asarray(feat, np.float32))
    src = np.asarray(src).astype(np.int64)
    dst = np.asarray(dst).astype(np.int64)
    N, D = feat.shape
    assert D == 128 and N % NCORES == 0

    meta, idx_all, rel_all, norm_src, norm_dst = _host_prep(N, D, src, dst)
    rpc, NT = meta["rpc"], meta["NT"]
    rpad = NT * P

    table16 = (feat * norm_src[:, None]).astype(np.float16)

    import jax
    with jax.default_device(jax.devices("cpu")[0]):
        keep = jax.random.bernoulli(jax.random.key(42), 0.5, (N, D))
        maskval = (np.asarray(keep).astype(np.float32)) * 2.0

    nc = _build_program(meta)

    in_maps = []
    for c in range(NCORES):
        nd = np.ones(rpad, np.float32)
        nd[:rpc] = norm_dst[c * rpc : (c + 1) * rpc]
        ndT = nd.reshape(NT, P).T.copy()           # [P, NT] col t = tile t
        ninv_a = (1.0 / nd).reshape(NT, P).copy()  # [NT, P]
        mrows = np.zeros((rpad, D), np.float32)
        mrows[:rpc] = maskval[c * rpc : (c + 1) * rpc]
        in_maps.append({
            "table": table16,
            "idxs": np.ascontiguousarray(idx_all[c]),
            "drel": np.ascontiguousarray(rel_all[c]),
            "ndstT": np.ascontiguousarray(ndT),
            "ninv": np.ascontiguousarray(ninv_a),
            "maskv": mrows,
            "W": np.ascontiguousarray(np.asarray(W, np.float32)),
            "b": np.ascontiguousarray(np.asarray(b, np.float32)),
            "gamma": np.ascontiguousarray(np.asarray(gamma, np.float32)),
            "beta": np.ascontiguousarray(np.asarray(beta, np.float32)),
        })

    res = run_bass_kernel_spmd(nc, in_maps, list(range(NCORES)))
    out = np.concatenate([res.results[c]["y"][:rpc] for c in range(NCORES)], axis=0)
    return out


# revision 13
# speedup vs baseline: 2.4120x; 1.5231x over previous
"""GCN layer (GraphConv + BN + dropout) as a Trainium2 Bass kernel, SPMD over 8 NeuronCores.

v2 design:
- dst-node sharding across 8 cores; gather table = feat * norm_src (folded on host,
  fp16) replicated per core; edges partitioned by (dst-tile, src-bank), padded to
  128-edge chunks with counts uniform across cores (max) so one NEFF runs SPMD.
- aggregation: dma_gather (4 SWDGE queues) + per-chunk 0/1 one-hot S (bf16, exact)
  matmul-accumulated into PSUM: aggT[f, d] += G^T S.
- stage B per tile: PSUM = invnorm_dst*b (K=1 matmul) + agg @ W, then
  h = relu(norm_dst * PSUM) via activation per-partition scale. BN stats via
  ones-matmul accumulation, all-reduced across cores; apply + dropout mask on DVE.
"""
import sys

sys.path.insert(0, "/opt/trn_rl_repo")

import numpy as np
import ml_dtypes

import concourse.bass as bass
import concourse.bacc as bacc
import concourse.mybir as mybir
import concourse.tile as tile
from concourse.bass_utils import run_bass_kernel_spmd

NCORES = 8
P = 128
BANK = 32768          # rows addressable by int16 gather index
OPC = 16              # max chunks (of 128 rows) per dma_gather op
NQ = 4                # SWDGE queues to rotate gather ops over
BN_EPS = 1e-5


def _host_prep(N, D, src, dst):
    """Partition edges by (core, dst-tile, src-bank); pad to 128-edge chunks with
    counts uniform across cores (max), producing per-core streams."""
    rpc = N // NCORES                      # rows per core
    NT = (rpc + P - 1) // P                # dst tiles per core
    NB = (N + BANK - 1) // BANK            # src banks

    deg_out = np.bincount(src, minlength=N).astype(np.float64)
    deg_in = np.bincount(dst, minlength=N).astype(np.float64)
    norm_src = (np.where(deg_out > 0, deg_out, 1.0) ** -0.5).astype(np.float32)
    norm_dst = (np.where(deg_in > 0, deg_in, 1.0) ** -0.5).astype(np.float32)

    core_of = dst // rpc
    ldst = dst - core_of * rpc
    tile_of = ldst // P
    rel_of = (ldst % P).astype(np.float32)
    bank_of = src >> 15
    lsrc = (src & (BANK - 1)).astype(np.int16)
    seg_of = tile_of * NB + bank_of        # segment id within a core

    NSEG = NT * NB
    per_core = []
    L = np.zeros((NCORES, NSEG), np.int64)
    for c in range(NCORES):
        m = core_of == c
        order = np.argsort(seg_of[m], kind="stable")
        idx = np.nonzero(m)[0][order]
        per_core.append((lsrc[idx], rel_of[idx]))
        L[c] = np.bincount(seg_of[m], minlength=NSEG)

    n_chunks = (L.max(axis=0) + P - 1) // P        # [NSEG], uniform over cores
    seg_pad = n_chunks * P
    total_edges = int(seg_pad.sum())
    TC = total_edges // P                          # total chunks per core

    ops = []  # (bank, nidx, chunk_base)
    k = 0
    for s in range(NSEG):
        nc_s = int(n_chunks[s])
        if nc_s == 0:
            continue
        b = s % NB
        done = 0
        while done < nc_s:
            take = min(OPC, nc_s - done)
            ops.append((b, take * P, k))
            k += take
            done += take
    assert k == TC

    idx_w = total_edges // 16
    idx_all = np.zeros((NCORES, 128, idx_w), np.int16)
    rel_all = np.full((NCORES, P, TC), -1.0, np.float32)
    seg_off = np.zeros(NSEG + 1, np.int64)
    np.cumsum(seg_pad, out=seg_off[1:])
    for c in range(NCORES):
        ls, rel = per_core[c]
        src_pad = np.zeros(total_edges, np.int16)
        rel_pad = np.full(total_edges, -1.0, np.float32)
        off = np.cumsum(np.concatenate([[0], L[c]]))[:-1]
        for s in range(NSEG):
            n = int(L[c, s])
            if n == 0:
                continue
            dstart = int(seg_off[s])
            src_pad[dstart : dstart + n] = ls[off[s] : off[s] + n]
            rel_pad[dstart : dstart + n] = rel[off[s] : off[s] + n]
        rel_all[c] = rel_pad.reshape(TC, P).T
        pos = 0
        for (_b, nidx, kb) in ops:
            seg = src_pad[kb * P : kb * P + nidx]
            blk = seg.reshape(-1, 16).T          # [16, nidx/16]
            idx_all[c, :, pos : pos + nidx // 16] = np.tile(blk, (8, 1))
            pos += nidx // 16
        assert pos == idx_w

    meta = dict(N=N, D=D, rpc=rpc, NT=NT, NB=NB, TC=TC, idx_w=idx_w,
                n_chunks=n_chunks, ops=ops, total_edges=total_edges)
    return meta, idx_all, rel_all, norm_src, norm_dst


def _build_program(meta, repeat_main=1, use_collective=True):
    N, D = meta["N"], meta["D"]
    NT, NB, TC = meta["NT"], meta["NB"], meta["TC"]
    idx_w, ops, n_chunks = meta["idx_w"], meta["ops"], meta["n_chunks"]
    rpad = NT * P
    nvalid_last = meta["rpc"] - (NT - 1) * P       # valid rows in last tile
    f32 = mybir.dt.float32
    bf16 = mybir.dt.bfloat16
    fp16 = mybir.dt.float16

    nc = bacc.Bacc("TRN2", num_devices=NCORES, num_swdge_queues=NQ)
    table = nc.declare_dram_parameter("table", [N, D], fp16, isOutput=False)
    idxs = nc.declare_dram_parameter("idxs", [128, idx_w], mybir.dt.int16, isOutput=False)
    drel = nc.declare_dram_parameter("drel", [P, TC], f32, isOutput=False)
    ndstT = nc.declare_dram_parameter("ndstT", [P, NT], f32, isOutput=False)
    ninv = nc.declare_dram_parameter("ninv", [NT, P], f32, isOutput=False)
    maskv = nc.declare_dram_parameter("maskv", [rpad, D], f32, isOutput=False)
    Wp = nc.declare_dram_parameter("W", [D, D], f32, isOutput=False)
    bp = nc.declare_dram_parameter("b", [D], f32, isOutput=False)
    gp = nc.declare_dram_parameter("gamma", [D], f32, isOutput=False)
    btp = nc.declare_dram_parameter("beta", [D], f32, isOutput=False)
    yout = nc.declare_dram_parameter("y", [rpad, D], f32, isOutput=True)

    cc_in = nc.dram_tensor("cc_in", [1, 2 * D], f32)
    cc_out = nc.dram_tensor("cc_out", [1, 2 * D], f32, addr_space="Shared")

    iota_np = np.tile(np.arange(P).astype(ml_dtypes.bfloat16), (P, 1))
    iota_dram = nc.inline_tensor(iota_np, name="iota_const")
    ones_last_np = np.zeros((P, 1), np.float32)
    ones_last_np[:nvalid_last] = 1.0
    ones_last_dram = nc.inline_tensor(ones_last_np, name="ones_last_const")

    ops_by_tile = [[] for _ in range(NT)]
    chunk_tile = np.repeat(np.arange(NT * NB) // NB, n_chunks)
    pos = 0
    for (b, nidx, kb) in ops:
        ops_by_tile[int(chunk_tile[kb])].append((b, nidx, kb, pos))
        pos += nidx // 16

    with tile.TileContext(nc) as tc:
        with tc.tile_pool(name="consts", bufs=1) as cpool, \
             tc.tile_pool(name="streams", bufs=1) as stpool, \
             tc.tile_pool(name="hbuf", bufs=1) as hpool, \
             tc.tile_pool(name="gat", bufs=10) as gpool, \
             tc.tile_pool(name="sel", bufs=24) as spool, \
             tc.tile_pool(name="aggsb", bufs=3) as apool, \
             tc.tile_pool(name="sq", bufs=2) as qpool, \
             tc.tile_pool(name="ybuf", bufs=4) as ypool, \
             tc.tile_pool(name="msk", bufs=4) as mpool, \
             tc.tile_pool(name="stat", bufs=1) as tpool, \
             tc.tile_pool(name="apsum", bufs=3, space="PSUM") as apsum, \
             tc.tile_pool(name="hpsum", bufs=2, space="PSUM") as hpsum, \
             tc.tile_pool(name="spsum", bufs=1, space="PSUM") as spsum:

            iota_sb = cpool.tile([P, P], bf16)
            nc.sync.dma_start(out=iota_sb[:], in_=iota_dram[:])
            W_sb = cpool.tile([D, D], f32)
            nc.sync.dma_start(out=W_sb[:], in_=Wp[:])
            b_row = cpool.tile([1, D], f32)
            nc.sync.dma_start(out=b_row[:], in_=bp[None, :])
            g_row = cpool.tile([1, D], f32)
            nc.sync.dma_start(out=g_row[:], in_=gp[None, :])
            bt_row = cpool.tile([1, D], f32)
            nc.sync.dma_start(out=bt_row[:], in_=btp[None, :])
            ones_col = cpool.tile([P, 1], f32)
            nc.vector.memset(ones_col[:], 1.0)
            ones_last = cpool.tile([P, 1], f32)
            nc.sync.dma_start(out=ones_last[:], in_=ones_last_dram[:])
            ones_row = cpool.tile([1, P], f32)
            nc.vector.memset(ones_row[:], 1.0)

            idx_sb = stpool.tile([128, idx_w], mybir.dt.int16)
            nc.sync.dma_start(out=idx_sb[:], in_=idxs[:])
            drel_sb = stpool.tile([P, TC], f32)
            nc.sync.dma_start(out=drel_sb[:], in_=drel[:])
            ndstT_sb = stpool.tile([P, NT], f32)
            nc.sync.dma_start(out=ndstT_sb[:], in_=ndstT[:])
            ninv_sb = stpool.tile([1, NT * P], f32)
            nc.sync.dma_start(out=ninv_sb[:], in_=ninv[:].rearrange("t p -> (t p)")[None, :])

            h_sb = hpool.tile([P, rpad], f32)

            ssum_ps = spsum.tile([1, D], f32, tag="ssum", name="ssum")
            ssq_ps = spsum.tile([1, D], f32, tag="ssq", name="ssq")

            opi = 0
            for rpt in range(repeat_main):
              for t in range(NT):
                agg_ps = apsum.tile([D, P], f32, tag="agg", name=f"agg{t}")
                nchunks_t = sum(nidx // P for (_b, nidx, _kb, _po) in ops_by_tile[t])
                done = 0
                for (bk, nidx, kb, po) in ops_by_tile[t]:
                    g = gpool.tile([128, nidx], fp16, tag="g", name=f"g{t}_{kb}")
                    bank_lo = bk * BANK
                    bank_hi = min(bank_lo + BANK, N)
                    nc.gpsimd.dma_gather(
                        g[:].rearrange("p (c e) -> p c e", e=D),
                        table[bank_lo:bank_hi, :],
                        idx_sb[:, po : po + nidx // 16],
                        nidx, nidx, D,
                        single_packet=(nidx <= 1024), queue_num=opi % NQ,
                    )
                    opi += 1
                    for j in range(nidx // P):
                        k = kb + j
                        import os as _os
                        if _os.environ.get("ABLATE_S") == "1":
                            S = iota_sb
                        else:
                            S = spool.tile([P, P], bf16, tag="S", name=f"S{k}")
                            nc.vector.tensor_scalar(
                                out=S[:], in0=iota_sb[:],
                                scalar1=drel_sb[:, k : k + 1],
                                scalar2=None,
                                op0=mybir.AluOpType.is_equal,
                            )
                        nc.tensor.matmul(
                            out=agg_ps[:],
                            lhsT=g[:, j * D : (j + 1) * D],
                            rhs=S[:],
                            start=(done == 0), stop=(done == nchunks_t - 1),
                        )
                        done += 1

                # stage B: PSUM = ninv*b + agg @ W ; h = relu(ndst * PSUM)
                aggT = apool.tile([D, P], f32, tag="aggT", name=f"aggT{t}")
                nc.scalar.copy(out=aggT[:], in_=agg_ps[:])
                h_ps = hpsum.tile([P, D], f32, tag="hT", name=f"hT{t}")
                nc.tensor.matmul(out=h_ps[:], lhsT=ninv_sb[:, t * P : (t + 1) * P],
                                 rhs=b_row[:], start=True, stop=False)
                nc.tensor.matmul(out=h_ps[:], lhsT=aggT[:], rhs=W_sb[:],
                                 start=False, stop=True)
                hslice = h_sb[:, t * P : (t + 1) * P]
                nc.scalar.activation(
                    out=hslice, in_=h_ps[:],
                    func=mybir.ActivationFunctionType.Relu,
                    scale=ndstT_sb[:, t : t + 1],
                )
                ones_t = ones_last if t == NT - 1 else ones_col
                nc.tensor.matmul(out=ssum_ps[:], lhsT=ones_t[:], rhs=hslice,
                                 start=(t == 0 and rpt == 0),
                                 stop=(t == NT - 1 and rpt == repeat_main - 1),
                                 skip_group_check=True)
                sq = qpool.tile([P, D], f32, tag="sq", name=f"sq{t}")
                nc.scalar.activation(out=sq[:], in_=hslice,
                                     func=mybir.ActivationFunctionType.Square)
                nc.tensor.matmul(out=ssq_ps[:], lhsT=ones_t[:], rhs=sq[:],
                                 start=(t == 0 and rpt == 0),
                                 stop=(t == NT - 1 and rpt == repeat_main - 1),
                                 skip_group_check=True)

            # BN stats -> all-reduce -> A/B rows -> broadcast
            stats = tpool.tile([1, 2 * D], f32)
            nc.vector.tensor_copy(out=stats[:, 0:D], in_=ssum_ps[:])
            nc.vector.tensor_copy(out=stats[:, D : 2 * D], in_=ssq_ps[:])
            nc.sync.dma_start(out=cc_in[:], in_=stats[:])
            if use_collective:
                nc.gpsimd.collective_compute(
                    "AllReduce", mybir.AluOpType.add,
                    replica_groups=[list(range(NCORES))],
                    ins=[cc_in[:]], outs=[cc_out[:]],
                )
            else:
                nc.sync.dma_start(out=cc_out[:], in_=cc_in[:])
            gstats = tpool.tile([1, 2 * D], f32)
            nc.sync.dma_start(out=gstats[:], in_=cc_out[:])

            inv_n = 1.0 / float(N)
            mean = tpool.tile([1, D], f32)
            nc.vector.tensor_scalar_mul(out=mean[:], in0=gstats[:, 0:D], scalar1=inv_n)
            ex2 = tpool.tile([1, D], f32)
            nc.vector.tensor_scalar_mul(out=ex2[:], in0=gstats[:, D : 2 * D], scalar1=inv_n)
            m2 = tpool.tile([1, D], f32)
            nc.vector.tensor_mul(out=m2[:], in0=mean[:], in1=mean[:])
            vare = tpool.tile([1, D], f32)
            nc.vector.tensor_sub(out=vare[:], in0=ex2[:], in1=m2[:])
            nc.vector.tensor_scalar_add(out=vare[:], in0=vare[:], scalar1=BN_EPS)
            rvar = tpool.tile([1, D], f32)
            nc.vector.reciprocal(out=rvar[:], in_=vare[:])
            rstd = tpool.tile([1, D], f32)
            nc.scalar.activation(out=rstd[:], in_=rvar[:],
                                 func=mybir.ActivationFunctionType.Sqrt)
            AB_row = tpool.tile([1, 2 * D], f32)
            nc.vector.tensor_mul(out=AB_row[:, 0:D], in0=g_row[:], in1=rstd[:])
            mA = tpool.tile([1, D], f32)
            nc.vector.tensor_mul(out=mA[:], in0=mean[:], in1=AB_row[:, 0:D])
            nc.vector.tensor_sub(out=AB_row[:, D : 2 * D], in0=bt_row[:], in1=mA[:])
            AB_ps = spsum.tile([P, 2 * D], f32, tag="ABps", name="ABps")
            nc.tensor.matmul(out=AB_ps[:], lhsT=ones_row[:], rhs=AB_row[:],
                             start=True, stop=True, skip_group_check=True)
            AB_sb = tpool.tile([P, 2 * D], f32)
            nc.vector.tensor_copy(out=AB_sb[:], in_=AB_ps[:])

            # apply BN + dropout mask, write out
            for rpt in range(repeat_main):
              for t in range(NT):
                yt = ypool.tile([P, D], f32, tag="yt", name=f"yt{t}")
                nc.vector.tensor_mul(out=yt[:], in0=h_sb[:, t * P : (t + 1) * P],
                                     in1=AB_sb[:, 0:D])
                nc.vector.tensor_add(out=yt[:], in0=yt[:], in1=AB_sb[:, D : 2 * D])
                mk = mpool.tile([P, D], f32, tag="mk", name=f"mk{t}")
                nc.sync.dma_start(out=mk[:], in_=maskv[t * P : (t + 1) * P, :])
                nc.vector.tensor_mul(out=yt[:], in0=yt[:], in1=mk[:])
                nc.sync.dma_start(out=yout[t * P : (t + 1) * P, :], in_=yt[:])

    nc.compile()
    return nc


def kernel(feat, src, dst, W, b, gamma, beta):
    feat = np.ascontiguousarray(np.asarray(feat, np.float32))
    src = np.asarray(src).astype(np.int64)
    dst = np.asarray(dst).astype(np.int64)
    N, D = feat.shape
    assert D == 128 and N % NCORES == 0

    meta, idx_all, rel_all, norm_src, norm_dst = _host_prep(N, D, src, dst)
    rpc, NT = meta["rpc"], meta["NT"]
    rpad = NT * P

    table16 = (feat * norm_src[:, None]).astype(np.float16)

    import jax
    with jax.default_device(jax.devices("cpu")[0]):
        keep = jax.random.bernoulli(jax.random.key(42), 0.5, (N, D))
        maskval = (np.asarray(keep).astype(np.float32)) * 2.0

    nc = _build_program(meta)

    in_maps = []
    for c in range(NCORES):
        nd = np.ones(rpad, np.float32)
        nd[:rpc] = norm_dst[c * rpc : (c + 1) * rpc]
        ndT = nd.reshape(NT, P).T.copy()           # [P, NT] col t = tile t
        ninv_a = (1.0 / nd).reshape(NT, P).copy()  # [NT, P]
        mrows = np.zeros((rpad, D), np.float32)
        mrows[:rpc] = maskval[c * rpc : (c + 1) * rpc]
        in_maps.append({
            "table": table16,
            "idxs": np.ascontiguousarray(idx_all[c]),
            "drel": np.ascontiguousarray(rel_all[c]),
            "ndstT": np.ascontiguousarray(ndT),
            "ninv": np.ascontiguousarray(ninv_a),
            "maskv": mrows,
            "W": np.ascontiguousarray(np.asarray(W, np.float32)),
            "b": np.ascontiguousarray(np.asarray(b, np.float32)),
            "gamma": np.ascontiguousarray(np.asarray(gamma, np.float32)),
            "beta": np.ascontiguousarray(np.asarray(beta, np.float32)),
        })

    res = run_bass_kernel_spmd(nc, in_maps, list(range(NCORES)))
    out = np.concatenate([res.results[c]["y"][:rpc] for c in range(NCORES)], axis=0)
    return out
